# revision 1
# baseline (speedup 1.0000x reference)
"""Trainium2 Bass kernel for nn_CIE_Net (GNN message passing + Sinkhorn).

Data-parallel over batch B=8 across 8 NeuronCores; each core runs the full
CIE-Net forward for one batch element. Channel-major edge layout [C, (i,j)];
fp32r for the large matmuls, fp16 for the layer-2 edge spill, fp32 for the
affinity/sinkhorn tail (validated end-to-end to 8.4e-7 vs the reference).
"""
import numpy as np

B, N, H = 8, 64, 1024
NCHUNK = H // 128          # 8 channel chunks
NIJ = N * N                # 4096 edge slots
TAU = 0.05
NEG = -1e30

_CACHE = {}


# --------------------------------------------------------------------------
# device program
# --------------------------------------------------------------------------

def _build(iters, dev=False):
    import concourse.bacc as bacc
    import concourse.mybir as mybir
    import concourse.tile as tile

    F32 = mybir.dt.float32
    F32R = mybir.dt.float32r
    F16 = mybir.dt.float16
    AX = mybir.AxisListType
    ALU = mybir.AluOpType
    ACTF = mybir.ActivationFunctionType

    class _Bacc(bacc.Bacc):
        # All activation funcs we emit (Exp, Ln, Relu, Identity, Copy) live in
        # the 'natural_log_exp_and_others' set; restricting the chooser to it
        # collapses ~34 LoadActFuncSet instructions (~1.3us each) into one.
        def insert_act_table_loads(self):
            import bass_rust as _br
            has_activation = any(
                isinstance(i, mybir.InstActivation)
                for b in self.main_func.blocks
                for i in b.instructions
            )
            if not has_activation:
                return
            keep = 'natural_log_exp_and_others'
            tabs = bacc.get_activation_tables(self.m.arch)
            tables = [(nm, (s if nm == keep else set()))
                      for nm, s in tabs.items()]
            _br.insert_act_table_loads(self, tables)

    nc = _Bacc("TRN2", target_bir_lowering=False, debug=False,
               enable_asserts=True, num_devices=8)

    def dram_in(name, shape, dt=F32):
        return nc.dram_tensor(name, list(shape), dt, kind="ExternalInput").ap()

    # ---- external inputs (per core) ----
    din = {}
    for g in (1, 2):
        din[f'fn{g}'] = dram_in(f'fn{g}', [N, H])
        din[f'ferep{g}'] = dram_in(f'ferep{g}', [128, NIJ])
        din[f'arep{g}'] = dram_in(f'arep{g}', [128, NIJ])
        din[f'afta{g}'] = dram_in(f'afta{g}', [N + 1, N])
        din[f'ata{g}'] = dram_in(f'ata{g}', [N + 1, N])
    for li in range(3):
        din[f'nw{li}'] = dram_in(f'nw{li}', [H, H], F16)
        din[f'sw{li}'] = dram_in(f'sw{li}', [H, H], F16)
        din[f'nb{li}'] = dram_in(f'nb{li}', [H])
        din[f'sb{li}'] = dram_in(f'sb{li}', [H])
        din[f'eb{li}'] = dram_in(f'eb{li}', [H])
    din['ew0v'] = dram_in('ew0v', [H])
    din['ew1h'] = dram_in('ew1h', [H, H], F16)
    din['ew2h'] = dram_in('ew2h', [H, H], F16)
    din['aff1'] = dram_in('aff1', [H, H], F16)
    din['aff2'] = dram_in('aff2', [H, H], F16)
    din['crossw'] = dram_in('crossw', [2 * H, H], F16)
    din['crossb'] = dram_in('crossb', [H], F16)
    for nm in ('maskA', 'negmaskA', 'maskB', 'negmaskB', 'maskotA'):
        din[nm] = dram_in(nm, [N, N])
    din['tbv'] = dram_in('tbv', [N, 1])
    din['tbinv'] = dram_in('tbinv', [N, 1])
    din['ident64'] = dram_in('ident64', [N, N])
    din['bias_all'] = dram_in('bias_all', [128, 80])
    din['ones64r'] = dram_in('ones64r', [1, N], F16)
    din['ident128'] = dram_in('ident128', [128, 128])
    din['ident128h'] = dram_in('ident128h', [128, 128], F16)

    out_s = nc.dram_tensor('out_s', [N, N], F32, kind="ExternalOutput").ap()

    dev_state = {}

    # internal DRAM spill for e2 (layer-1 edge output), fp16, per graph
    e2sp = {g: nc.dram_tensor(f'e2sp{g}', [NCHUNK, 128, NIJ], F16).ap()
            for g in (1, 2)}

    with tile.TileContext(nc) as tc:
        import contextlib
        ctx = contextlib.ExitStack()
        with ctx:
            sbuf_pool = ctx.enter_context(tc.tile_pool(name="sbuf", bufs=1))
            psum_pool = ctx.enter_context(tc.tile_pool(name="psum", bufs=1,
                                                       space="PSUM"))

            class _Router:
                """single pool; per-tag bufs via tile(bufs=)"""
                BUFS = {'echunk': 2, 'nxT': 2, 'rsxT': 2, 'row64': 2, 't1T': 1,
                        'nxa': 2, 'ebsw': 2,
                        'feslc': 2, 'aslc': 2, 'echunk_': 0,
                        'sk_ls': 2, 'sk_ls1': 1, 'sk_e': 1, 'sk_a': 1,
                        'sk_b': 1, 'sk_sp': 1, 'sk_spT': 1, 'sk_eA': 1,
                        'sk_eB': 1, 'sk_oa': 1, 'sk_ob': 1, 'sk_out': 1,
                        'sk_outT': 1, 'agg': 2,
                        'wsl': 8, 'e2st': 4, 'tA': 4, 'wmn': 4, 'l0a': 2,
                        'exch': 2,
                        'l0b': 2, 'ragg': 4, 'sk_rmax': 2, 'sk_nrmax': 2,
                        'sk_ss': 2, 'sk_lg': 2}
                _n = 0

                def __init__(self, pool, extra):
                    self.pool = pool
                    self.extra = extra

                def tile(self, shape, dtype, tag=None):
                    if tag is None:
                        _Router._n += 1
                        tag = f"p{_Router._n}"
                    bufs = {**self.BUFS, **self.extra}.get(tag, 1)
                    return self.pool.tile(shape, dtype, name=tag, tag=tag,
                                          bufs=bufs)

            sb = _Router(sbuf_pool, {})
            ps = _Router(psum_pool, {'edge': 2, 'node': 2, 'tr': 2})

            dma = nc.sync.dma_start

            def dump(name, src_ap):
                """dev-mode: immediately DMA an SBUF AP to a debug output."""
                if not dev:
                    return
                shape = list(src_ap.shape)
                d = nc.dram_tensor(f'dev_{name}', shape, src_ap.dtype,
                                   kind="ExternalOutput").ap()
                dma(d, src_ap)

            # ============ prologue: constants ============
            t_ident64 = sb.tile([N, N], F32); dma(t_ident64[:], din['ident64'])
            t_ident128 = sb.tile([128, 128], F32); dma(t_ident128[:], din['ident128'])
            t_ident128h = sb.tile([128, 128], F16)
            dma(t_ident128h[:], din['ident128h'])
            # ============ helpers ============
            def transpose_64x128_chunks(src_row, n_chunks, out_tile, act_bias=None,
                                        act_func=ACTF.Identity, copy_engine='scalar'):
                """src_row: [64, n_chunks*128] SBUF -> out_tile [128, n_chunks*64]
                channel-major chunks, optional per-partition bias at the copy."""
                for k in range(n_chunks):
                    p_tr = ps.tile([128, N], F32, tag="tr")
                    nc.tensor.transpose(p_tr[:], src_row[:, k*128:(k+1)*128],
                                        t_ident64[:])
                    dst = out_tile[:, k*N:(k+1)*N]
                    if act_bias is not None:
                        nc.scalar.activation(dst, p_tr[:], act_func,
                                             bias=act_bias[:, k:k+1], scale=1.0)
                    elif act_func != ACTF.Identity or copy_engine == 'scalar':
                        nc.scalar.activation(dst, p_tr[:], act_func)
                    else:
                        nc.vector.tensor_copy(dst, p_tr[:])

            def node_matmuls_2g(lhs_by_g, w_name, out_by_g):
                """Row-major node matmul for both graphs sharing weight slices:
                out_by_g[g] [64, 1024] = emb_g @ W."""
                for h in range(2):
                    p_nx = {g: ps.tile([N, 512], F32, tag="node")
                            for g in (1, 2)}
                    for k in range(NCHUNK):
                        t_w = sb.tile([128, 512], F16, tag="wsl")
                        dma(t_w[:], din[w_name][k*128:(k+1)*128, h*512:(h+1)*512])
                        for g in (1, 2):
                            nc.tensor.matmul(p_nx[g][:],
                                             lhs_by_g[g][:, k*N:(k+1)*N], t_w[:],
                                             start=(k == 0),
                                             stop=(k == NCHUNK - 1))
                    for g in (1, 2):
                        nc.scalar.activation(out_by_g[g][:, h*512:(h+1)*512],
                                             p_nx[g][:], ACTF.Copy)

            def full_transpose_64(dst, src):
                """64x64 full transpose via 4 StreamTranspose block ops (DVE)."""
                for bi in range(2):
                    for bj in range(2):
                        nc.vector.transpose(dst[bi*32:(bi+1)*32, bj*32:(bj+1)*32],
                                            src[bj*32:(bj+1)*32, bi*32:(bi+1)*32])

            # ============ sinkhorn ============
            def sinkhorn(s_psum, want_transpose_out, dbg=None):
                """s_psum: [64,64] fp32 PSUM affinity scores. Returns (s_sb, sT_sb)
                in s-coords; sT_sb None unless want_transpose_out."""
                sp = sb.tile([N, N], F32, tag="sk_sp")
                nc.vector.tensor_copy(sp[:], s_psum[:])
                if dbg is not None:
                    dump(f'spre{dbg}', sp[:])
                spT = sb.tile([N, N], F32, tag="sk_spT")
                full_transpose_64(spT[:], sp[:])
                # blend: s_sel = tbinv*s + tbv*sT  (A-orientation start)
                a_t = sb.tile([N, N], F32, tag="sk_a")
                b_t = sb.tile([N, N], F32, tag="sk_b")
                nc.vector.tensor_scalar(a_t[:], sp[:], t_tbinv[:], None, op0=ALU.mult)
                nc.vector.tensor_scalar(b_t[:], spT[:], t_tbv[:], None, op0=ALU.mult)
                ls = sb.tile([N, N], F32, tag="sk_ls")
                nc.vector.tensor_tensor(ls[:], a_t[:], b_t[:], op=ALU.add)
                nc.vector.tensor_tensor(ls[:], ls[:], t_masks['maskotA'][:], op=ALU.mult)
                nc.vector.tensor_tensor(ls[:], ls[:], t_masks['negmaskA'][:], op=ALU.add)
                for it in range(iters):
                    mA = t_masks['maskA'] if it % 2 == 0 else t_masks['maskB']
                    nmA = t_masks['negmaskA'] if it % 2 == 0 else t_masks['negmaskB']
                    rmax = sb.tile([N, 1], F32, tag="sk_rmax")
                    nrmax = sb.tile([N, 1], F32, tag="sk_nrmax")
                    nc.vector.tensor_reduce(rmax[:], ls[:], axis=AX.X, op=ALU.max)
                    nc.vector.tensor_reduce(nrmax[:], ls[:], axis=AX.X, op=ALU.max,
                                            negate=True)
                    e_t = sb.tile([N, N], F32, tag="sk_e")
                    ssum = sb.tile([N, 1], F32, tag="sk_ss")
                    nc.scalar.activation(e_t[:], ls[:], ACTF.Exp, bias=nrmax[:],
                                         scale=1.0, accum_out=ssum[:])
                    lg = sb.tile([N, 1], F32, tag="sk_lg")
                    nc.scalar.activation(lg[:], ssum[:], ACTF.Ln, bias=0.0, scale=1.0)
                    ls1 = sb.tile([N, N], F32, tag="sk_ls1")
                    nc.vector.tensor_scalar(ls1[:], ls[:], rmax[:], lg[:],
                                            op0=ALU.subtract, op1=ALU.subtract)
                    nc.vector.tensor_tensor(ls1[:], ls1[:], mA[:], op=ALU.mult)
                    nc.vector.tensor_tensor(ls1[:], ls1[:], nmA[:], op=ALU.add)
                    ls = sb.tile([N, N], F32, tag="sk_ls")
                    full_transpose_64(ls[:], ls1[:])
                # ls is in orientation A if iters is even, else B
                o = iters % 2
                mO = t_masks['maskA'] if o == 0 else t_masks['maskB']
                eO = sb.tile([N, N], F32, tag="sk_eA")
                nc.scalar.activation(eO[:], ls[:], ACTF.Exp, bias=0.0, scale=1.0)
                nc.vector.tensor_tensor(eO[:], eO[:], mO[:], op=ALU.mult)
                eOther = sb.tile([N, N], F32, tag="sk_eB")
                full_transpose_64(eOther[:], eO[:])
                eA, eB = (eO, eOther) if o == 0 else (eOther, eO)
                # out = tb ? eB : eA
                oa = sb.tile([N, N], F32, tag="sk_oa")
                ob = sb.tile([N, N], F32, tag="sk_ob")
                nc.vector.tensor_scalar(oa[:], eA[:], t_tbinv[:], None, op0=ALU.mult)
                nc.vector.tensor_scalar(ob[:], eB[:], t_tbv[:], None, op0=ALU.mult)
                s_out = sb.tile([N, N], F32, tag="sk_out")
                nc.vector.tensor_tensor(s_out[:], oa[:], ob[:], op=ALU.add)
                sT_out = None
                if want_transpose_out:
                    sT_out = sb.tile([N, N], F32, tag="sk_outT")
                    full_transpose_64(sT_out[:], s_out[:])
                return s_out, sT_out

            # ============ per-graph state ============
            embT1 = {}   # [128, 512] f32r, post-layer1 embeddings (pre-cross)
            embT2 = {}   # post-cross
            embT3 = {}   # post-layer2
            agg_t = {}

            # ============ phase: load fn, build fnT ============
            fnT = {}
            for g in (1, 2):
                fn_row = sb.tile([N, H], F32, tag="row64")
                dma(fn_row[:], din[f'fn{g}'])
                fnT[g] = sb.tile([128, NCHUNK * N], F16, tag=f"fnT{g}")
                transpose_64x128_chunks(fn_row, NCHUNK, fnT[g])

            def edge_layer(g, li, nxa_l, nxT_l, rsxT_l, emb_out_tag,
                           early_drain_n=-1):
                """Shared L1/L2 edge path with eb folded via SW matmul.
                li==1: e1 from ferep on the fly, spill e2. li==2: reload e2."""
                ebn = f'eb{li}'
                # SW[c,i] = sum_j A[i,j]*(nx+nb)[c,j]; ebsw = eb[c]*SW
                ebsw = sb.tile([128, NCHUNK * N], F32, tag="ebsw")
                for m in range(NCHUNK):
                    p_sw = ps.tile([128, N], F32, tag="tr")
                    nc.tensor.matmul(p_sw[:], nxa_l[:, m*128:(m+1)*128],
                                     t_ata[g][:], start=True, stop=True)
                    nc.vector.tensor_scalar(ebsw[:, m*N:(m+1)*N], p_sw[:],
                                            t_bias[ebn][:, m:m+1], None,
                                            op0=ALU.mult)
                agg = sb.tile([128, NCHUNK * N], F32, tag="agg")
                agg_t[g] = agg
                NP = 2  # n-chunks per psum group (2 banks)
                for n2 in range(NCHUNK // NP):
                    n0 = n2 * NP
                    t_asl = sb.tile([128, 512 * NP], F32, tag="aslc")
                    dma(t_asl[:], din[f'arep{g}'][:, n0*512:(n0+NP)*512])
                    if li == 1:
                        t_fesl = sb.tile([128, 512 * NP], F32, tag="feslc")
                        dma(t_fesl[:], din[f'ferep{g}'][:, n0*512:(n0+NP)*512])
                        # layout: k-chunks of [128, NP*512], fp16
                        en = sb.tile([128, NCHUNK * NP * 512], F16, tag="echunk")
                        for k in range(NCHUNK):
                            nc.scalar.activation(
                                en[:, k*512*NP:(k+1)*512*NP], t_fesl[:],
                                ACTF.Relu,
                                bias=t_bias['eb0'][:, k:k+1],
                                scale=t_bias['ew0v'][:, k:k+1])
                        t_ew = t_ew1
                    else:
                        en = sb.tile([128, NCHUNK * NP * 512], F16, tag="echunk")
                        for k in range(NCHUNK):
                            dma(en[:, k*512*NP:(k+1)*512*NP],
                                e2sp[g][k, :, n0*512:(n0+NP)*512])
                        t_ew = t_ew2
                    for m in range(NCHUNK):
                        p_e = ps.tile([128, 512 * NP], F32, tag="edge")
                        for j in range(NP):
                            for k in range(NCHUNK):
                                nc.tensor.matmul(
                                    p_e[:, j*512:(j+1)*512],
                                    t_ew[:, k*H + m*128: k*H + (m+1)*128],
                                    en[:, k*512*NP + j*512: k*512*NP + (j+1)*512],
                                    start=(k == 0), stop=(k == NCHUNK - 1))
                        if li == 1:
                            e2st = sb.tile([128, 512 * NP], F16, tag="e2st")
                            nc.scalar.activation(e2st[:], p_e[:], ACTF.Relu,
                                                 bias=t_bias['eb1'][:, m:m+1],
                                                 scale=1.0)
                            dma(e2sp[g][m, :, n0*512:(n0+NP)*512], e2st[:])
                        if n2 <= early_drain_n:
                            exch = sb.tile([128, 512 * NP], F32, tag="exch")
                            nc.scalar.activation(exch[:], p_e[:], ACTF.Copy)
                            p_e = exch
                        wmn = sb.tile([128, 512 * NP], F32, tag="wmn")
                        nc.gpsimd.tensor_tensor(
                            wmn[:].rearrange("p (i j) -> p i j", j=N),
                            t_asl[:].rearrange("p (i j) -> p i j", j=N),
                            nxT_l[:, m*N:(m+1)*N].unsqueeze(1).broadcast_to(
                                [128, NCHUNK * NP, N]),
                            op=ALU.mult)
                        tA = sb.tile([128, 512 * NP], F32, tag="tA")
                        nc.vector.tensor_tensor(tA[:], p_e[:], wmn[:], op=ALU.mult)
                        nc.vector.tensor_reduce(
                            agg[:, m*N + n0*NCHUNK: m*N + (n0+NP)*NCHUNK],
                            tA[:].rearrange("p (i j) -> p i j", j=N),
                            axis=AX.X, op=ALU.add)
                emb_new = sb.tile([128, NCHUNK * N], F16, tag=emb_out_tag)
                for m in range(NCHUNK):
                    t_r = sb.tile([128, N], F32, tag="ragg")
                    nc.vector.tensor_tensor(t_r[:], agg[:, m*N:(m+1)*N],
                                            ebsw[:, m*N:(m+1)*N], op=ALU.add)
                    nc.scalar.activation(t_r[:], t_r[:], ACTF.Relu)
                    nc.vector.tensor_tensor(emb_new[:, m*N:(m+1)*N], t_r[:],
                                            rsxT_l[:, m*N:(m+1)*N], op=ALU.add)
                return emb_new

            # ============ L0 (both graphs, shared weight slices) ============
            nxa0 = {g: sb.tile([N + 1, H], F32, tag="nxa") for g in (1, 2)}
            node_matmuls_2g(fnT, 'nw0', {g: nxa0[g][0:N, :] for g in (1, 2)})
            sx0 = {g: sb.tile([N, H], F32, tag="row64") for g in (1, 2)}
            node_matmuls_2g(fnT, 'sw0', sx0)

            # constants (emitted late; SWDGE queue keeps HWDGE clear)
            t_masks = {}
            for nm in ('maskA', 'negmaskA', 'maskB', 'negmaskB', 'maskotA'):
                t = sb.tile([N, N], F32, tag=f"m_{nm}")
                nc.gpsimd.dma_start(t[:], din[nm]); t_masks[nm] = t
            t_tbv = sb.tile([N, 1], F32); nc.gpsimd.dma_start(t_tbv[:], din['tbv'])
            t_tbinv = sb.tile([N, 1], F32); nc.gpsimd.dma_start(t_tbinv[:], din['tbinv'])
            t_ones_r = sb.tile([1, N], F16)
            nc.gpsimd.dma_start(t_ones_r[:], din['ones64r'])

            BIAS_ORDER = ['nb0', 'sb0', 'eb0', 'nb1', 'sb1', 'eb1',
                          'nb2', 'sb2', 'eb2', 'ew0v']
            t_bias_all = sb.tile([128, 80], F32)
            nc.gpsimd.dma_start(t_bias_all[:], din['bias_all'])
            t_bias = {nm: t_bias_all[:, vi*NCHUNK:(vi+1)*NCHUNK]
                      for vi, nm in enumerate(BIAS_ORDER)}
            t_crossb = sb.tile([1, H], F16)
            nc.gpsimd.dma_start(t_crossb[:], din['crossb'].unsqueeze(0))
            t_afta = {g: sb.tile([N + 1, N], F32, tag=f"afta{g}") for g in (1, 2)}
            t_ata = {g: sb.tile([N + 1, N], F32, tag=f"ata{g}") for g in (1, 2)}
            for g in (1, 2):
                nc.gpsimd.dma_start(t_afta[g][:], din[f'afta{g}'])
                nc.gpsimd.dma_start(t_ata[g][:], din[f'ata{g}'])

            for g in (1, 2):
                dma(nxa0[g][N:N+1, :], din['nb0'].unsqueeze(0))
                rsxT = sb.tile([128, NCHUNK * N], F32, tag="rsxT")
                transpose_64x128_chunks(sx0[g], NCHUNK, rsxT,
                                        act_bias=t_bias['sb0'], act_func=ACTF.Relu)
                embT1[g] = sb.tile([128, NCHUNK * N], F16, tag=f"embT1_{g}")
                for m in range(NCHUNK):
                    p_m1 = ps.tile([128, N], F32, tag="tr")
                    nc.tensor.matmul(p_m1[:], nxa0[g][:, m*128:(m+1)*128],
                                     t_afta[g][:], start=True, stop=True)
                    p_m2 = ps.tile([128, N], F32, tag="tr")
                    nc.tensor.matmul(p_m2[:], nxa0[g][:, m*128:(m+1)*128],
                                     t_ata[g][:], start=True, stop=True)
                    t_a = sb.tile([128, N], F32, tag="l0a")
                    t_b = sb.tile([128, N], F32, tag="l0b")
                    nc.vector.tensor_scalar(t_a[:], p_m1[:],
                                            t_bias['ew0v'][:, m:m+1], None,
                                            op0=ALU.mult)
                    nc.vector.tensor_scalar(t_b[:], p_m2[:],
                                            t_bias['eb0'][:, m:m+1], None,
                                            op0=ALU.mult)
                    nc.vector.tensor_tensor(t_a[:], t_a[:], t_b[:], op=ALU.add)
                    nc.scalar.activation(t_a[:], t_a[:], ACTF.Relu)
                    nc.vector.tensor_tensor(embT1[g][:, m*N:(m+1)*N], t_a[:],
                                            rsxT[:, m*N:(m+1)*N], op=ALU.add)

            # ============ L1 node (both graphs) ============
            nxa1 = {g: sb.tile([N + 1, H], F32, tag="nxa") for g in (1, 2)}
            node_matmuls_2g(embT1, 'nw1', {g: nxa1[g][0:N, :] for g in (1, 2)})
            sx1 = {g: sb.tile([N, H], F32, tag="row64") for g in (1, 2)}
            node_matmuls_2g(embT1, 'sw1', sx1)
            nxT1 = {}
            rsxT1 = {}
            for g in (1, 2):
                dma(nxa1[g][N:N+1, :], din['nb1'].unsqueeze(0))
                nxT1[g] = sb.tile([128, NCHUNK * N], F32, tag="nxT")
                transpose_64x128_chunks(nxa1[g][0:N, :], NCHUNK, nxT1[g],
                                        act_bias=t_bias['nb1'])
                rsxT1[g] = sb.tile([128, NCHUNK * N], F32, tag="rsxT")
                transpose_64x128_chunks(sx1[g], NCHUNK, rsxT1[g],
                                        act_bias=t_bias['sb1'], act_func=ACTF.Relu)

            # ============ L1 edge per graph ============
            t_ew1 = sb.tile([128, NCHUNK * H], F16, tag="wmat")
            for k in range(NCHUNK):
                dma(t_ew1[:, k*H:(k+1)*H], din['ew1h'][k*128:(k+1)*128, :])
            t_ew2 = None
            for g in (1, 2):
                embT1[g] = edge_layer(g, 1, nxa1[g], nxT1[g], rsxT1[g],
                                      f"embT1n_{g}")

            dump('embT1_g1', embT1[1][:])
            dump('embT1_g2', embT1[2][:])
            dump('agg1_g1', agg_t[1][:])

            # ============ affinity1 + sinkhorn1 ============
            def affinity(embT_a, embT_b, aff_name):
                """s_pre [64,64] psum = (a^T A b) via row-major t1 + transpose."""
                t1row = sb.tile([N, H], F32, tag="row64")
                for h in range(2):
                    p_t1 = ps.tile([N, 512], F32, tag="node")
                    for k in range(NCHUNK):
                        t_w = sb.tile([128, 512], F16, tag="wsl")
                        dma(t_w[:], din[aff_name][k*128:(k+1)*128,
                                                  h*512:(h+1)*512])
                        nc.tensor.matmul(p_t1[:], embT_a[:, k*N:(k+1)*N], t_w[:],
                                         start=(k == 0), stop=(k == NCHUNK - 1))
                    nc.scalar.activation(t1row[:, h*512:(h+1)*512], p_t1[:],
                                         ACTF.Copy)
                t1T = sb.tile([128, NCHUNK * N], F32, tag="t1T")
                transpose_64x128_chunks(t1row, NCHUNK, t1T, copy_engine='vector')
                emb_b32 = sb.tile([128, NCHUNK * N], F32, tag="embb32")
                nc.vector.tensor_copy(emb_b32[:], embT_b[:])
                p_sp = ps.tile([N, N], F32, tag="tr")
                for k in range(NCHUNK):
                    nc.tensor.matmul(p_sp[:], t1T[:, k*N:(k+1)*N],
                                     emb_b32[:, k*N:(k+1)*N],
                                     start=(k == 0), stop=(k == NCHUNK - 1))
                return p_sp

            p_sp1 = affinity(embT1[1], embT1[2], 'aff1')
            s1, s1T = sinkhorn(p_sp1, want_transpose_out=True, dbg='1')
            dump('s1', s1[:])

            # load ew2 now: overlaps sinkhorn/cross
            t_ew2 = sb.tile([128, NCHUNK * H], F16, tag="wmat")
            for k in range(NCHUNK):
                dma(t_ew2[:, k*H:(k+1)*H], din['ew2h'][k*128:(k+1)*128, :])

            # ============ cross layer ============
            emb_row = {}
            for g in (1, 2):
                emb_row[g] = sb.tile([N, H], F32, tag=f"embrow{g}")
                for m in range(NCHUNK):
                    p_tr = ps.tile([N, 128], F16, tag="tr")
                    nc.tensor.transpose(p_tr[:],
                                        embT1[g][:, m*N:(m+1)*N],
                                        t_ident128h[:])
                    nc.vector.tensor_copy(emb_row[g][:, m*128:(m+1)*128], p_tr[:])
            # uT[c,i] = sum_j emb2_row[j,c] * s1T[j,i];  vT with emb1_row, s1
            uT = sb.tile([128, NCHUNK * N], F16, tag="uT")
            vT = sb.tile([128, NCHUNK * N], F16, tag="vT")
            for m in range(NCHUNK):
                p_u = ps.tile([128, N], F32, tag="tr")
                nc.tensor.matmul(p_u[:], emb_row[2][:, m*128:(m+1)*128], s1T[:],
                                 start=True, stop=True)
                nc.scalar.activation(uT[:, m*N:(m+1)*N], p_u[:], ACTF.Copy)
                p_v = ps.tile([128, N], F32, tag="tr")
                nc.tensor.matmul(p_v[:], emb_row[1][:, m*128:(m+1)*128], s1[:],
                                 start=True, stop=True)
                nc.scalar.activation(vT[:, m*N:(m+1)*N], p_v[:], ACTF.Copy)

            cat = {1: (embT1[1], uT), 2: (embT1[2], vT)}
            emb_c_row = {g: sb.tile([N, H], F32, tag=f"embcrow{g}")
                         for g in (1, 2)}
            for h in range(2):
                p_c = {g: ps.tile([N, 512], F32, tag="node") for g in (1, 2)}
                for k in range(16):
                    t_w = sb.tile([128, 512], F16, tag="wsl")
                    dma(t_w[:], din['crossw'][k*128:(k+1)*128, h*512:(h+1)*512])
                    kk = k % NCHUNK
                    for g in (1, 2):
                        lhsT = (cat[g][0] if k < NCHUNK else cat[g][1])
                        nc.tensor.matmul(p_c[g][:], lhsT[:, kk*N:(kk+1)*N],
                                         t_w[:], start=(k == 0), stop=False)
                for g in (1, 2):
                    nc.tensor.matmul(p_c[g][:], t_ones_r[:],
                                     t_crossb[:, h*512:(h+1)*512],
                                     start=False, stop=True)
                    nc.scalar.activation(emb_c_row[g][:, h*512:(h+1)*512],
                                         p_c[g][:], ACTF.Copy)
            for g in (1, 2):
                embT2[g] = sb.tile([128, NCHUNK * N], F16, tag=f"embT2_{g}")
                transpose_64x128_chunks(emb_c_row[g], NCHUNK, embT2[g])

            dump('embT2_g1', embT2[1][:])

            # ============ L2 (node both graphs, then edge per graph) ============
            nxa2 = {g: sb.tile([N + 1, H], F32, tag="nxa") for g in (1, 2)}
            node_matmuls_2g(embT2, 'nw2', {g: nxa2[g][0:N, :] for g in (1, 2)})
            sx2 = {g: sb.tile([N, H], F32, tag="row64") for g in (1, 2)}
            node_matmuls_2g(embT2, 'sw2', sx2)
            nxT2 = {}
            rsxT2 = {}
            for g in (1, 2):
                dma(nxa2[g][N:N+1, :], din['nb2'].unsqueeze(0))
                nxT2[g] = sb.tile([128, NCHUNK * N], F32, tag="nxT")
                transpose_64x128_chunks(nxa2[g][0:N, :], NCHUNK, nxT2[g],
                                        act_bias=t_bias['nb2'])
                rsxT2[g] = sb.tile([128, NCHUNK * N], F32, tag="rsxT")
                transpose_64x128_chunks(sx2[g], NCHUNK, rsxT2[g],
                                        act_bias=t_bias['sb2'], act_func=ACTF.Relu)
            for g in (2, 1):
                embT3[g] = edge_layer(g, 2, nxa2[g], nxT2[g], rsxT2[g],
                                      f"embT3_{g}",
                                      early_drain_n=(0 if g == 2 else -1))

            dump('embT3_g1', embT3[1][:])

            # ============ affinity2 + sinkhorn2 -> output ============
            p_sp2 = affinity(embT3[1], embT3[2], 'aff2')
            s_fin, _ = sinkhorn(p_sp2, want_transpose_out=False, dbg='2')
            dma(out_s, s_fin[:])

    nc.compile()
    return nc


# --------------------------------------------------------------------------
# host side
# --------------------------------------------------------------------------

def _prep_core_inputs(inputs, b):
    """Build the per-core input map for batch element b."""
    f32 = np.float32
    g = lambda k: np.ascontiguousarray(np.asarray(inputs[k]), dtype=f32)
    m = {}
    for gi in (1, 2):
        fn = g(f'feat_node{gi}')[b]
        A = g(f'A{gi}')[b]
        fe = g(f'feat_edge{gi}')[b][:, :, 0]
        m[f'fn{gi}'] = fn
        m[f'ferep{gi}'] = np.broadcast_to(fe.reshape(1, -1), (128, NIJ)).copy()
        m[f'arep{gi}'] = np.broadcast_to(A.reshape(1, -1), (128, NIJ)).copy()
        AF = (A * fe).astype(f32)
        m[f'afta{gi}'] = np.vstack([AF.T, AF.sum(1)[None, :]]).astype(f32)
        m[f'ata{gi}'] = np.vstack([A.T, A.sum(1)[None, :]]).astype(f32)
    for li in range(3):
        m[f'nw{li}'] = g(f'l{li}_nw').astype(np.float16)
        m[f'sw{li}'] = g(f'l{li}_sw').astype(np.float16)
        m[f'nb{li}'] = g(f'l{li}_nb')
        m[f'sb{li}'] = g(f'l{li}_sb')
        m[f'eb{li}'] = g(f'l{li}_eb')
    m['ew0v'] = g('l0_ew').reshape(-1)
    m['ew1h'] = g('l1_ew').astype(np.float16)
    m['ew2h'] = g('l2_ew').astype(np.float16)
    m['aff1'] = g('aff1_A').astype(np.float16)
    m['aff2'] = g('aff2_A').astype(np.float16)
    m['crossw'] = g('cross_w').astype(np.float16)
    m['crossb'] = g('cross_b').astype(np.float16)
    n1 = int(np.asarray(inputs['n1'])[b])
    n2 = int(np.asarray(inputs['n2'])[b])
    tb = n1 > n2
    nr, ncol = (n2, n1) if tb else (n1, n2)
    rmask = (np.arange(N) < nr)[:, None]
    cmask = (np.arange(N) < ncol)[None, :]
    maskA = (rmask & cmask).astype(f32)
    m['maskA'] = maskA
    m['negmaskA'] = ((1 - maskA) * NEG).astype(f32)
    m['maskB'] = maskA.T.copy()
    m['negmaskB'] = (m['negmaskA'].T).copy()
    m['maskotA'] = (maskA / TAU).astype(f32)
    m['tbv'] = np.full((N, 1), 1.0 if tb else 0.0, f32)
    m['tbinv'] = np.full((N, 1), 0.0 if tb else 1.0, f32)
    m['ident64'] = np.eye(N, dtype=f32)
    m['ones64r'] = np.ones((1, N), np.float16)
    m['ident128h'] = np.eye(128, dtype=np.float16)
    border = ['nb0', 'sb0', 'eb0', 'nb1', 'sb1', 'eb1', 'nb2', 'sb2', 'eb2',
              'ew0v']
    ba = np.zeros((128, 80), f32)
    for vi, nm in enumerate(border):
        ba[:, vi*8:(vi+1)*8] = m[nm].reshape(8, 128).T
    m['bias_all'] = ba
    m['ident128'] = np.eye(128, dtype=f32)
    return m


def kernel(**inputs):
    import os
    from concourse.bass_utils import run_bass_kernel_spmd
    iters = int(np.asarray(inputs['sk_max_iter']))
    key = ('main', iters)
    if key not in _CACHE:
        _CACHE[key] = _build(iters)
    nc = _CACHE[key]
    in_maps = [_prep_core_inputs(inputs, b) for b in range(B)]
    kw = {}
    if os.environ.get('KERNEL_TRACE'):
        kw = dict(trace=True, stitch_traces=True,
                  trace_cores=list(range(B)))
    res = run_bass_kernel_spmd(nc, in_maps, core_ids=list(range(B)), **kw)
    if os.environ.get('KERNEL_TRACE'):
        kernel._last_results = res
    out = np.stack([res.results[b]['out_s'] for b in range(B)], axis=0)
    return out.astype(np.float32)


kernel._last_results = None

